# revision 1
# baseline (speedup 1.0000x reference)
"""Multi-head attention (B=2, S=2048, D=1024, H=16) on 8 trn2 NeuronCores.

Tensor-parallel over heads (2 heads per core, column-sliced wq/wk/wv) for the
QKV projections and attention; a per-(batch, head-group) AllToAll then
redistributes the attention output so each core computes the output
projection for its own interleaved 512-row slice of the flattened (B*S)
sequence (Megatron-style TP with a sequence-parallel output projection).

Layout/engine choices:
  - the host supplies x.T and w.T so every matmul operand arrives K-major;
    no activation transposes on device
  - logits are computed transposed [t, s] so the softmax exp (over t) feeds
    the P@V matmul directly -- no probability-matrix transposes
  - ones-columns appended to V produce the softmax denominators in the same
    PV matmul (PSUM rows 64..127), replicated across partitions for a cheap
    vector normalize
  - matmuls run in float32r (full-rate relaxed fp32); the x/w stream and the
    projection tail (attnT, collective buffers, wo) are float16
  - exp runs on ACT from 2x[128,1024] double-buffered PSUM logit tiles --
    ACT is the attention-phase bottleneck, PE fills gaps with PV/logit mms
  - attention processes s in two half-passes so it needs only 6 PSUM banks;
    the freed 2 banks let batch-1's QKV projections and V-transposes fold
    into batch-0's ACT-bound attention window (PE and ACT both ~95% busy)
  - the four 0.25MB AllToAlls overlap attention; both output projections
    run in the tail, overlapping the only exposed (last) collective
"""

import sys

sys.path.insert(0, "/opt/trn_rl_repo")

import numpy as np

import concourse.mybir as mybir
import concourse.tile as tile
from concourse import bacc
from concourse.bass_utils import run_bass_kernel_spmd
from concourse.masks import make_identity

B, S, D = 2, 2048, 1024
H, HD = 16, 64
NCORES = 8
DL = D // NCORES          # 128 local attn dims (2 heads) per core
R = B * S                 # 4096 flattened rows
RSL = R // NCORES         # 512 output rows per core
P = 128
KC = D // P               # 8 contraction chunks of 128
TC = S // P               # 16 key/t chunks per batch
SB = 512                  # moving-operand (N) tile
NSB = S // SB             # 4 s-chunks per batch
F32 = mybir.dt.float32
F32R = mybir.dt.float32r
F16 = mybir.dt.float16

_CACHE = {}


def _build(n_iters=1, phases=3, bench=False):
    nc = bacc.Bacc("TRN2", target_bir_lowering=False, debug=False,
                   num_devices=NCORES)
    Exp = mybir.ActivationFunctionType.Exp

    kind = "Internal" if bench else "ExternalInput"
    xT = nc.dram_tensor("xT", [D, R], F16, kind=kind)
    wqT = nc.dram_tensor("wqT", [D, DL], F16, kind=kind)
    wkT = nc.dram_tensor("wkT", [D, DL], F16, kind=kind)
    wvT = nc.dram_tensor("wvT", [D, DL], F16, kind=kind)
    woT = nc.dram_tensor("woT", [D, D], F16, kind=kind)
    bqkv = nc.dram_tensor("bqkv", [DL, 3], F32, kind=kind)
    bo_t = nc.dram_tensor("bo_t", [P, NCORES], F32, kind=kind)
    out = nc.dram_tensor("out", [D, RSL], F32, kind="ExternalOutput")

    with tile.TileContext(nc) as tc:
        with (
            tc.tile_pool(name="const", bufs=1) as const,
            tc.tile_pool(name="persist", bufs=1) as persist,
            tc.tile_pool(name="dram", bufs=1, space="DRAM") as dram,
        ):
            # ---- constants / weights resident in SBUF ----
            ident = const.tile([P, P], F16, tag="ident")
            make_identity(nc, ident[:])
            bias3 = const.tile([DL, 3], F32, tag="bias3")
            bo_s = const.tile([P, NCORES], F32, tag="bo_s")
            if bench:
                nc.vector.memset(bias3[:], 0.0)
                nc.vector.memset(bo_s[:], 0.0)
            else:
                nc.sync.dma_start(bias3[:], bqkv[:])
                nc.sync.dma_start(bo_s[:], bo_t[:])

            w_s = []
            for name in ("wq", "wk", "wv"):
                w_s.append(const.tile([P, D], F16, tag=f"w_{name}",
                                      name=f"w_{name}"))
            for t, wt in ((w_s[0], wqT),):
                if bench:
                    nc.vector.memset(t[:], 0.0)
                else:
                    nc.sync.dma_start(
                        t[:].rearrange("p (kc c) -> p kc c", c=P),
                        wt.rearrange("(kc p) c -> p kc c", p=P))
            wo_s = [const.tile([P, D], F16, tag=f"wo{kc}", name=f"wo{kc}")
                    for kc in range(KC)]

            # persistent activations
            QT = persist.tile([P, R], F32R, tag="QT")   # [2 heads*64, B*S]
            KT = persist.tile([P, R], F32R, tag="KT")
            VT = persist.tile([P, R], F16, tag="VT")
            # V natural per 128-row t-chunk: [v_h0 |ones| v_h1 |ones]
            vn = persist.tile([P, (R // P) * 256], F16, tag="vn")
            vn3 = vn[:].rearrange("p (g two c) -> p g two c", two=2, c=128)
            nc.vector.memset(vn3[:, :, :, 64:128], 1.0)
            attnT = persist.tile([P, R], F16, tag="attnT")

            for it in range(n_iters):
                SH = S // 2
                CW = RSL // 2
                a2a_in = [[dram.tile([NCORES, HD, CW], F16,
                                     tag=f"a2a_in{it}_{b}_{h}",
                                     name=f"a2a_in{it}_{b}_{h}")
                           for h in range(2)] for b in range(B)]
                a2a_out = [[dram.tile([NCORES, HD, CW], F16,
                                      tag=f"a2a_out{it}_{b}_{h}",
                                      name=f"a2a_out{it}_{b}_{h}")
                            for h in range(2)] for b in range(B)]

                def load_half(half, xt_pool):
                    hof = half * (R // 2)
                    xts = []
                    for kc in range(KC):
                        t = xt_pool.tile([P, R // 2], F16, tag="xt",
                                         name=f"xt_{it}_{half}_{kc}")
                        nc.sync.dma_start(
                            t[:], xT[kc * P:(kc + 1) * P, hof:hof + R // 2])
                        xts.append(t)
                        if it == 0 and half == 0 and kc == 0:
                            # wk/wv ride the queue behind the first x tile:
                            # the first q-matmuls only need wq + x[0]
                            for wtile, wt in ((w_s[1], wkT), (w_s[2], wvT)):
                                if bench:
                                    nc.vector.memset(wtile[:], 0.0)
                                else:
                                    nc.sync.dma_start(
                                        wtile[:].rearrange(
                                            "p (kc c) -> p kc c", c=P),
                                        wt.rearrange("(kc p) c -> p kc c",
                                                     p=P))
                    return xts

                def qkv_copy(pj, i, nb, hof, ps):
                    dst_ap = (QT, KT, VT)[pj][:, hof + nb * SB:
                                              hof + (nb + 1) * SB]
                    if (pj + i) % 2 == 0:
                        nc.vector.tensor_scalar_add(dst_ap, ps[:],
                                                    bias3[:, pj:pj + 1])
                    else:
                        nc.scalar.add(dst_ap, ps[:], bias3[:, pj:pj + 1])

                def vnat(half, pool, tag):
                    # V natural (+ ones) tiles for this half's t-chunks
                    for g in range(half * 16, half * 16 + 16):
                        pt = pool.tile([P, P], F16, tag=tag,
                                       name=f"pt_{it}_{half}_{g}")
                        nc.tensor.transpose(pt[:], VT[:, g * P:(g + 1) * P],
                                            ident[:])
                        o = g * 256
                        nc.vector.tensor_copy(vn[:, o:o + 64], pt[:, 0:64])
                        nc.vector.tensor_copy(vn[:, o + 128:o + 192],
                                              pt[:, 64:128])

                def attention_batch(b, ps3, exps, norm):
                    base = b * S
                    for h in range(2):
                        hr = slice(h * HD, (h + 1) * HD)
                        for sh in range(2):
                            sof = base + sh * SH
                            pv = ps3.tile([P, SH], F32, tag="pv", bufs=1,
                                          name=f"pv_{it}_{b}_{h}_{sh}")
                            for tcn in range(TC):
                                ex = exps.tile([P, SH], F16, tag="ex",
                                               name=f"ex_{it}_{b}_{h}_{sh}_{tcn}")
                                lg = ps3.tile([P, SH], F32, tag="lg", bufs=2,
                                              name=f"lg_{it}_{b}_{h}_{sh}_{tcn}")
                                for sb in range(2):
                                    nc.tensor.matmul(
                                        lg[:, sb * SB:(sb + 1) * SB],
                                        KT[hr, base + tcn * P:
                                           base + (tcn + 1) * P],
                                        QT[hr, sof + sb * SB:
                                           sof + (sb + 1) * SB],
                                        start=True, stop=True)
                                nc.scalar.activation(ex[:], lg[:], Exp,
                                                     scale=1.0 / 8.0)
                                o = (b * TC + tcn) * 256 + h * 128
                                for sb in range(2):
                                    nc.tensor.matmul(
                                        pv[:, sb * SB:(sb + 1) * SB],
                                        vn[:, o:o + 128],
                                        ex[:, sb * SB:(sb + 1) * SB],
                                        start=(tcn == 0), stop=(tcn == TC - 1))
                            vcp = norm.tile([P, SH], F32, tag="vcp")
                            nc.vector.tensor_copy(vcp[:], pv[:])
                            rc = norm.tile([HD, SH], F32, tag="rc")
                            nc.vector.reciprocal(rc[:], vcp[64:128, :])
                            nc.vector.tensor_mul(
                                attnT[h * HD:(h + 1) * HD, sof:sof + SH],
                                vcp[0:64, :], rc[:])
                        # ship this (batch, head) chunk; overlaps compute
                        if phases >= 3:
                            nc.sync.dma_start(
                                a2a_in[b][h].rearrange("j p c -> p j c"),
                                attnT[h * HD:(h + 1) * HD,
                                      base:base + S].rearrange(
                                          "p (j c) -> p j c", c=CW))
                            nc.gpsimd.collective_compute(
                                "AllToAll", mybir.AluOpType.bypass,
                                replica_groups=[list(range(NCORES))],
                                ins=[a2a_in[b][h].opt()],
                                outs=[a2a_out[b][h].opt()])

                def proj_batch(b, proj, ps4, outs):
                    rh_b = proj.tile([P, KC * CW], F16, tag=f"rh{it}_{b}",
                                     name=f"rh{it}_{b}")
                    for h in range(2):
                        nc.sync.dma_start(
                            rh_b[h * HD:(h + 1) * HD, :].rearrange(
                                "p (kc c) -> p kc c", c=CW),
                            a2a_out[b][h].rearrange("kc p c -> p kc c"))
                    for mc in range(KC):
                        ps = ps4.tile([P, CW], F32, tag="ps4",
                                      name=f"ps4_{it}_{b}_{mc}")
                        for kc in range(KC):
                            nc.tensor.matmul(
                                ps[:], wo_s[kc][:, mc * P:(mc + 1) * P],
                                rh_b[:, kc * CW:(kc + 1) * CW],
                                start=(kc == 0), stop=(kc == KC - 1))
                        ot = outs.tile([P, CW], F32, tag="ot",
                                       name=f"ot_{it}_{b}_{mc}")
                        nc.vector.tensor_scalar_add(ot[:], ps[:],
                                                    bo_s[:, mc:mc + 1])
                        nc.sync.dma_start(
                            out[mc * P:(mc + 1) * P, b * CW:(b + 1) * CW],
                            ot[:])

                with tc.tile_pool(name=f"xt{it}", bufs=8) as xt_pool:
                    # ---- batch-0 QKV + V-transposes (full-width PSUM) ----
                    with (
                        tc.tile_pool(name=f"ps1{it}", bufs=6,
                                     space="PSUM") as ps1,
                        tc.tile_pool(name=f"pst{it}", bufs=2,
                                     space="PSUM") as pst,
                    ):
                        xts0 = load_half(0, xt_pool)
                        for np_ in range(2):
                            pss = [[ps1.tile([P, SB], F32, tag="ps1",
                                             name=f"ps1_{it}_0_{np_}_{pj}_{i}")
                                    for i in range(2)] for pj in range(3)]
                            for kc in range(KC):
                                for pj in range(3):
                                    for i in range(2):
                                        nb = np_ * 2 + i
                                        nc.tensor.matmul(
                                            pss[pj][i][:],
                                            w_s[pj][:, kc * P:(kc + 1) * P],
                                            xts0[kc][:, nb * SB:(nb + 1) * SB],
                                            start=(kc == 0),
                                            stop=(kc == KC - 1))
                            for pj in range(3):
                                for i in range(2):
                                    qkv_copy(pj, i, np_ * 2 + i, 0,
                                             pss[pj][i])
                        vnat(0, pst, "pst")

                    for kc in range(KC):
                        if bench:
                            nc.vector.memset(wo_s[kc][:], 0.0)
                        else:
                            nc.sync.dma_start(
                                wo_s[kc][:], woT[kc * P:(kc + 1) * P, :])
                    if phases < 2:
                        continue

                    with (
                        tc.tile_pool(name=f"ps3{it}", bufs=1,
                                     space="PSUM") as ps3,
                        tc.tile_pool(name=f"exps{it}", bufs=4) as exps,
                        tc.tile_pool(name=f"norm{it}", bufs=2) as norm,
                    ):
                        # attention b0 (6 banks); QKV-half1 gap-fills PE
                        attention_batch(0, ps3, exps, norm)

                        with tc.tile_pool(name=f"ps1b{it}", bufs=2,
                                          space="PSUM") as ps1b:
                            xts1 = load_half(1, xt_pool)
                            hof = R // 2
                            for pj in range(3):
                                for nb in range(4):
                                    t = ps1b.tile([P, SB], F32, tag="ps1b",
                                                  name=f"ps1b_{it}_{pj}_{nb}")
                                    for kc in range(KC):
                                        nc.tensor.matmul(
                                            t[:],
                                            w_s[pj][:, kc * P:(kc + 1) * P],
                                            xts1[kc][:, nb * SB:(nb + 1) * SB],
                                            start=(kc == 0),
                                            stop=(kc == KC - 1))
                                    qkv_copy(pj, nb % 2, nb, hof, t)
                            vnat(1, ps1b, "ps1b")

                        attention_batch(1, ps3, exps, norm)

                if phases < 3:
                    continue
                with (
                    tc.tile_pool(name=f"proj1{it}", bufs=1) as proj1,
                    tc.tile_pool(name=f"ps41{it}", bufs=4,
                                 space="PSUM") as ps41,
                    tc.tile_pool(name=f"outs1{it}", bufs=4) as outs1,
                ):
                    proj_batch(0, proj1, ps41, outs1)
                    proj_batch(1, proj1, ps41, outs1)

    nc.compile()
    return nc


def _get_program(n_iters=1, phases=3, bench=False):
    key = (n_iters, phases, bench)
    if key not in _CACHE:
        _CACHE[key] = _build(n_iters, phases, bench)
    return _CACHE[key]


def _in_maps(x, wq, bq, wk, bk, wv, bv, wo, bo):
    x = np.asarray(x, np.float32)
    xT = np.ascontiguousarray(x.reshape(R, D).T.astype(np.float16))
    woT = np.ascontiguousarray(
        np.asarray(wo, np.float32).T.astype(np.float16))
    bo_t = np.ascontiguousarray(
        np.asarray(bo, np.float32).reshape(NCORES, P).T)
    maps = []
    for i in range(NCORES):
        sl = slice(i * DL, (i + 1) * DL)
        maps.append({
            "xT": xT,
            "wqT": np.ascontiguousarray(np.asarray(wq, np.float32)[sl, :].T
                                        .astype(np.float16)),
            "wkT": np.ascontiguousarray(np.asarray(wk, np.float32)[sl, :].T
                                        .astype(np.float16)),
            "wvT": np.ascontiguousarray(np.asarray(wv, np.float32)[sl, :].T
                                        .astype(np.float16)),
            "woT": woT,
            "bqkv": np.ascontiguousarray(np.stack(
                [np.asarray(bq, np.float32)[sl],
                 np.asarray(bk, np.float32)[sl],
                 np.asarray(bv, np.float32)[sl]], axis=1)),
            "bo_t": bo_t,
        })
    return maps


def kernel(x, wq, bq, wk, bk, wv, bv, wo, bo, **_):
    nc = _get_program()
    res = run_bass_kernel_spmd(nc, _in_maps(x, wq, bq, wk, bk, wv, bv, wo, bo),
                               list(range(NCORES)))
    # core j holds, for each batch b, output columns
    # [b*2048 + j*256, b*2048 + (j+1)*256) of out.T
    CW = RSL // 2
    outT = np.empty((D, R), np.float32)
    for j in range(NCORES):
        o = res.results[j]["out"]
        for b in range(B):
            outT[:, b * S + j * CW:(b * S) + (j + 1) * CW] = \
                o[:, b * CW:(b + 1) * CW]
    return np.ascontiguousarray(outT.T).reshape(B, S, D)



# revision 30
# speedup vs baseline: 1.1292x; 1.1292x over previous
"""Multi-head attention (B=2, S=2048, D=1024, H=16) on 8 trn2 NeuronCores.

Tensor-parallel over heads (2 heads per core, column-sliced wq/wk/wv) for the
QKV projections and attention; a per-(batch, s-half) AllToAll redistributes
the attention output so each core computes the output projection for its own
interleaved 128-col chunks of the flattened (B*S) sequence.

Engine plan: PE does all matmuls, ACT does only the softmax exp, DVE does all
psum->sbuf copies and the normalize. The Tile scheduler reorders by priority
(emission order) among dependency-ready work, so the code shapes the schedule
with priorities and explicit gates rather than literal instruction order:
  - x streams in nb-major ~1MB chunks and weights arrive host-preswizzled
    (contiguous [128, D] dmas) so the first K-projection matmul starts ~4us in
  - V is computed directly in natural [t, dl] layout (stationary x-chunks,
    moving wv) so no PE transposes are needed; bv folds into bo on the host;
    ones-columns interleaved with V produce the softmax denominators inside
    the PV matmuls; normalize reads PV psum directly (reciprocal+mul on DVE)
  - logits are computed transposed [t, s]; the whole lg->exp->pv->normalize
    chunk body runs at high priority so QKV filler work never starves the
    ACT-paced attention pipeline; a deep (12-buf) exp ring lets PV lag while
    V tiles arrive
  - batch-1 K/Q(s0) drain into batch-0's attention window as PE filler
    (they gate batch-1 logits); V-b1/Q-b1(s1) drain into batch-1's window
  - AllToAlls fire per (batch, s-half) as soon as both heads normalize (the
    two heads write separate attnT tiles so the ship dmas see precise deps);
    the three early projection passes are dependency-gated behind the last
    normalize so they fill the PE under the final collective, leaving only
    one 128-col projection pass after it
"""

import sys

sys.path.insert(0, "/opt/trn_rl_repo")

import numpy as np

import concourse.mybir as mybir
import concourse.tile as tile
from concourse import bacc
from concourse.bass_utils import run_bass_kernel_spmd

B, S, D = 2, 2048, 1024
H, HD = 16, 64
NCORES = 8
DL = D // NCORES          # 128 local attn dims (2 heads) per core
R = B * S                 # 4096 flattened rows
RSL = R // NCORES         # 512 output rows per core
P = 128
KC = D // P               # 8 contraction chunks of 128
TC = S // P               # 16 key/t chunks per batch
SB = 512                  # moving-operand (N) tile for QKV / logits
NB = S // SB              # 4 nb chunks per batch half of x
SH = S // 2               # 1024-wide s half
CW = 128                  # per-core output column chunk (per batch, sh)
F32 = mybir.dt.float32
F32R = mybir.dt.float32r
F16 = mybir.dt.float16

_CACHE = {}


def _build(n_iters=1, phases=3, bench=False):
    nc = bacc.Bacc("TRN2", target_bir_lowering=False, debug=False,
                   num_devices=NCORES)
    Exp = mybir.ActivationFunctionType.Exp

    kind = "Internal" if bench else "ExternalInput"
    xT = nc.dram_tensor("xT", [D, R], F16, kind=kind)
    wqT = nc.dram_tensor("wqT", [P, D], F16, kind=kind)
    wkT = nc.dram_tensor("wkT", [P, D], F16, kind=kind)
    wvT = nc.dram_tensor("wvT", [P, D], F16, kind=kind)
    woT = nc.dram_tensor("woT", [P, KC * D], F16, kind=kind)
    bqk = nc.dram_tensor("bqk", [DL, 2], F32, kind=kind)
    bo_t = nc.dram_tensor("bo_t", [P, KC], F32, kind=kind)
    out = nc.dram_tensor("out", [D, RSL], F32, kind="ExternalOutput")

    with tile.TileContext(nc) as tc:
        with (
            tc.tile_pool(name="const", bufs=1) as const,
            tc.tile_pool(name="persist", bufs=1) as persist,
            tc.tile_pool(name="dram", bufs=1, space="DRAM") as dram,
        ):
            bias2 = const.tile([DL, 2], F32, tag="bias2")
            bo_s = const.tile([P, KC], F32, tag="bo_s")
            w_s = {}
            for name in ("wq", "wk", "wv"):
                w_s[name] = const.tile([P, D], F16, tag=f"w_{name}",
                                       name=f"w_{name}")
            wo_all = const.tile([P, KC * D], F16, tag="wo_all")
            wo_s = [wo_all[:, kc * D:(kc + 1) * D] for kc in range(KC)]

            def load_w(t, wt):
                if bench:
                    nc.vector.memset(t[:], 0.0)
                else:
                    nc.sync.dma_start(t[:], wt[:, :])

            # wk first: the K projection runs first so exp starts early
            load_w(w_s["wk"], wkT)

            # persistent activations
            QT = persist.tile([P, R], F32R, tag="QT")   # [2 heads*64, B*S]
            KT = persist.tile([P, R], F32R, tag="KT")
            # V natural per 128-row t-chunk: [v_h0 |ones| v_h1 |ones]
            vn = persist.tile([P, (R // P) * 256], F16, tag="vn")
            vn3 = vn[:].rearrange("p (g two c) -> p g two c", two=2, c=128)
            nc.vector.memset(vn3[:, :, :, 64:128], 1.0)
            attnTs = [persist.tile([P, R], F16, tag=f"attnT{h}",
                                   name=f"attnT{h}") for h in range(2)]

            for it in range(n_iters):
                a2a_in = [[dram.tile([NCORES, P, CW], F16,
                                     tag=f"a2a_in{it}_{b}_{sh}",
                                     name=f"a2a_in{it}_{b}_{sh}")
                           for sh in range(2)] for b in range(B)]
                a2a_out = [[dram.tile([NCORES, P, CW], F16,
                                      tag=f"a2a_out{it}_{b}_{sh}",
                                      name=f"a2a_out{it}_{b}_{sh}")
                            for sh in range(2)] for b in range(B)]

                with tc.tile_pool(name=f"xt{it}", bufs=2) as xt_pool:
                    xth = [xt_pool.tile([P, KC * S], F16, tag="xt",
                                        name=f"xt_{it}_{half}")
                           for half in range(2)]

                    def xs(half, kc):
                        return xth[half][:, kc * S:(kc + 1) * S]

                    def load_x(half):
                        # nb-major so the first 1MB arrives fast; one
                        # strided DMA per nb covers all kc chunks
                        hof = half * S
                        xv = xth[half][:].rearrange("p (kc s) -> p kc s", s=S)
                        srcv = xT.rearrange("(kc p) r -> p kc r", p=P)
                        for nb in range(NB):
                            nsplit = 4 if (half == 0 and nb == 0) else 2
                            for kh in range(nsplit):
                                kq = KC // nsplit
                                ks = slice(kh * kq, (kh + 1) * kq)
                                nc.sync.dma_start(
                                    xv[:, ks, nb * SB:(nb + 1) * SB],
                                    srcv[:, ks,
                                         hof + nb * SB:hof + (nb + 1) * SB])
                            if half == 0 and nb == 0:
                                load_w(w_s["wq"], wqT)
                                load_w(w_s["wv"], wvT)
                                if bench:
                                    nc.vector.memset(bias2[:], 0.0)
                                    nc.vector.memset(bo_s[:], 0.0)
                                else:
                                    nc.sync.dma_start(bias2[:], bqk[:])
                                    nc.sync.dma_start(bo_s[:], bo_t[:])

                    def qk_group(pj, half, nb, pool, tag):
                        # one [128dl, 512rows] psum group + DVE copy w/ bias
                        name, dst = (("wk", KT), ("wq", QT))[pj]
                        ps = pool.tile([P, SB], F32, tag=tag,
                                       name=f"qk_{it}_{half}_{pj}_{nb}")
                        w = w_s[name]
                        for kc in range(KC):
                            nc.tensor.matmul(
                                ps[:], w[:, kc * P:(kc + 1) * P],
                                xs(half, kc)[:, nb * SB:(nb + 1) * SB],
                                start=(kc == 0), stop=(kc == KC - 1))
                        o = half * S + nb * SB
                        nc.vector.tensor_scalar_add(
                            dst[:, o:o + SB], ps[:],
                            bias2[:, 1 - pj:2 - pj])

                    def v_group(half, q, pool, tag):
                        # natural-V for 4 t-chunks: psum [128t, 4*128dl]
                        ps = pool.tile([P, SB], F32, tag=tag,
                                       name=f"vq_{it}_{half}_{q}")
                        wv = w_s["wv"]
                        for i in range(4):
                            tcn = q * 4 + i
                            for kc in range(KC):
                                nc.tensor.matmul(
                                    ps[:, i * P:(i + 1) * P],
                                    xs(half, kc)[:, tcn * P:(tcn + 1) * P],
                                    wv[:, kc * P:(kc + 1) * P],
                                    start=(kc == 0), stop=(kc == KC - 1))
                        for i in range(4):
                            g = half * TC + q * 4 + i
                            o = g * 256
                            nc.vector.tensor_copy(
                                vn[:, o:o + 64], ps[:, i * P:i * P + 64])
                            nc.vector.tensor_copy(
                                vn[:, o + 128:o + 192],
                                ps[:, i * P + 64:(i + 1) * P])

                    norm_muls = {}

                    def attention_batch(b, ps3, exps, norm):
                        base = b * S
                        for sh in range(2):
                            for h in range(2):
                                hr = slice(h * HD, (h + 1) * HD)
                                sof = base + sh * SH
                                pv = ps3.tile([P, SH], F32, tag="pv", bufs=1,
                                              name=f"pv_{it}_{b}_{h}_{sh}")
                                for tcn in range(TC):
                                    ex = exps.tile(
                                        [P, SH], F16, tag="ex",
                                        name=f"ex_{it}_{b}_{h}_{sh}_{tcn}")
                                    lg = ps3.tile(
                                        [P, SH], F32, tag="lg", bufs=2,
                                        name=f"lg_{it}_{b}_{h}_{sh}_{tcn}")
                                    with tc.high_priority(offset=400):
                                        for sb in range(2):
                                            nc.tensor.matmul(
                                                lg[:, sb * SB:(sb + 1) * SB],
                                                KT[hr, base + tcn * P:
                                                   base + (tcn + 1) * P],
                                                QT[hr, sof + sb * SB:
                                                   sof + (sb + 1) * SB],
                                                start=True, stop=True)
                                        nc.scalar.activation(
                                            ex[:], lg[:], Exp,
                                            scale=1.0 / 8.0)
                                        o = (b * TC + tcn) * 256 + h * 128
                                        for sb in range(2):
                                            nc.tensor.matmul(
                                                pv[:, sb * SB:(sb + 1) * SB],
                                                vn[:, o:o + 128],
                                                ex[:, sb * SB:
                                                   (sb + 1) * SB],
                                                start=(tcn == 0),
                                                stop=(tcn == TC - 1))
                                # normalize straight out of PV psum,
                                # per sb half so it pipelines with pv mms
                                rc = norm.tile([HD, SH], F32, tag="rc")
                                nq, qw = 2, SB
                                with tc.high_priority(offset=400):
                                    for sb in range(nq):
                                        sl = slice(sb * qw, (sb + 1) * qw)
                                        nc.vector.reciprocal(
                                            rc[:, sl], pv[64:128, sl])
                                        norm_muls[(b, sh, h, sb)] = \
                                            nc.vector.tensor_mul(
                                                attnTs[h][0:HD,
                                                          sof + sb * qw:
                                                          sof + (sb + 1) * qw],
                                                pv[0:64, sl], rc[:, sl])
                            # ship this (batch, s-half); overlaps compute
                            if phases >= 3:
                                stk = tc.high_priority(offset=400)
                                stk.__enter__()
                                for h2 in (1, 0):
                                    hs = slice(h2 * HD, (h2 + 1) * HD)
                                    nc.sync.dma_start(
                                        a2a_in[b][sh].rearrange(
                                            "j p c -> p j c")[hs],
                                        attnTs[h2][0:HD,
                                                   base + sh * SH:
                                                   base + (sh + 1) * SH]
                                        .rearrange("p (j c) -> p j c", c=CW))
                                nc.gpsimd.collective_compute(
                                    "AllToAll", mybir.AluOpType.bypass,
                                    replica_groups=[list(range(NCORES))],
                                    ins=[a2a_in[b][sh].opt()],
                                    outs=[a2a_out[b][sh].opt()])
                                stk.__exit__(None, None, None)

                    outv = out.rearrange("(mc p) c -> p mc c", p=P)

                    def proj_pass(b, sh, rh, ncols, coff, ps4, outs):
                        # 8 mc-chunks + one batched out DMA
                        ot = outs.tile([P, KC * ncols], F32, tag="ot",
                                       name=f"ot_{it}_{b}_{sh}")
                        for mc in range(KC):
                            ps = ps4.tile([P, SB], F32, tag="aux",
                                          name=f"ps4_{it}_{b}_{sh}_{mc}")
                            for kc in range(KC):
                                nc.tensor.matmul(
                                    ps[:, 0:ncols],
                                    wo_s[kc][:, mc * P:(mc + 1) * P],
                                    rh[:, kc * ncols:(kc + 1) * ncols],
                                    start=(kc == 0), stop=(kc == KC - 1))
                            nc.vector.tensor_scalar_add(
                                ot[:, mc * ncols:(mc + 1) * ncols],
                                ps[:, 0:ncols], bo_s[:, mc:mc + 1])
                        for mh in range(2):
                            ms = slice(mh * (KC // 2), (mh + 1) * (KC // 2))
                            nc.sync.dma_start(
                                outv[:, ms, coff:coff + ncols],
                                ot[:].rearrange("p (mc c) -> p mc c",
                                                c=ncols)[:, ms])

                    # ---- head: QKV-b0, K first, V natural; the attn
                    # psum pools coexist so the first logits/exp can be
                    # hoisted under the head by the scheduler ----
                    with (
                        tc.tile_pool(name=f"ps3{it}", bufs=1,
                                     space="PSUM") as ps3,
                        tc.tile_pool(name=f"exps{it}", bufs=12) as exps,
                        tc.tile_pool(name=f"norm{it}", bufs=2) as norm,
                        tc.tile_pool(name=f"aux{it}", bufs=2,
                                     space="PSUM") as aux,
                    ):
                        load_x(0)
                        qk_group(0, 0, 0, aux, "aux")  # K nb0
                        qk_group(1, 0, 0, aux, "aux")  # Q nb0
                        qk_group(1, 0, 1, aux, "aux")  # Q nb1
                        qk_group(0, 0, 1, aux, "aux")  # K nb1
                        qk_group(0, 0, 2, aux, "aux")  # K nb2
                        qk_group(0, 0, 3, aux, "aux")  # K nb3
                        for q in range(4):
                            v_group(0, q, aux, "aux")
                        qk_group(1, 0, 2, aux, "aux")  # Q nb2
                        qk_group(1, 0, 3, aux, "aux")  # Q nb3
                        load_x(1)

                        if bench:
                            nc.vector.memset(wo_all[:], 0.0)
                        else:
                            nc.sync.dma_start(wo_all[:], woT[:, :])
                        if phases < 2:
                            continue

                        attention_batch(0, ps3, exps, norm)

                        # fillers drain into attention's PE slack:
                        # K/Q-s0 of b1 gate attn-b1's logits; V-b1 and
                        # Q-s1-b1 defer into the b1 window
                        for pj, nb in ((0, 0), (1, 0), (1, 1), (0, 1),
                                       (0, 2), (0, 3)):
                            qk_group(pj, 1, nb, aux, "aux")
                        for q in range(4):
                            v_group(1, q, aux, "aux")
                        qk_group(1, 1, 2, aux, "aux")
                        qk_group(1, 1, 3, aux, "aux")

                        if phases < 3:
                            continue

                        attention_batch(1, ps3, exps, norm)

                        # output projection, one pass per (batch, s-half).
                        # rh DMAs gated (dep=) so the scheduler cannot hoist
                        # dependent matmuls before the collective really
                        # lands; ungated passes hoist into attn-b1 PE gaps.
                        with (
                            tc.tile_pool(name=f"proj{it}", bufs=1) as proj,
                            tc.tile_pool(name=f"outs{it}", bufs=2) as outs,
                        ):
                            # all but the last pass run under the final
                            # collective: gate their rh loads on the last
                            # norm so the scheduler cannot pull the matmuls
                            # into the (already saturated) attention windows
                            import bass_rust as _br
                            last_norm = norm_muls[(1, 1, 1, 1)]
                            for b in range(B):
                                for sh in range(2):
                                    rh = proj.tile([P, KC * CW], F16,
                                                   tag=f"rh{it}_{b}_{sh}",
                                                   name=f"rh{it}_{b}_{sh}")
                                    for kh in range(2):
                                        ks = slice(kh * (KC // 2),
                                                   (kh + 1) * (KC // 2))
                                        dma = nc.sync.dma_start(
                                            rh[:].rearrange(
                                                "p (kc c) -> p kc c",
                                                c=CW)[:, ks],
                                            a2a_out[b][sh].rearrange(
                                                "kc p c -> p kc c")[:, ks])
                                        if (b, sh) != (1, 1):
                                            dma.ins.add_dependency(
                                                last_norm.ins.name,
                                                _br.DependencyInfo(
                                                    sync=True,
                                                    no_sync=False))
                                    proj_pass(b, sh, rh, CW,
                                              b * 2 * CW + sh * CW,
                                              aux, outs)
    nc.compile()
    return nc


def _get_program(n_iters=1, phases=3, bench=False):
    key = (n_iters, phases, bench)
    if key not in _CACHE:
        _CACHE[key] = _build(n_iters, phases, bench)
    return _CACHE[key]


def _in_maps(x, wq, bq, wk, bk, wv, bv, wo, bo):
    x = np.asarray(x, np.float32)
    xT = np.ascontiguousarray(x.reshape(R, D).T.astype(np.float16))
    wo32 = np.asarray(wo, np.float32)
    # device layout [P, KC*D]: woT_sw[p, kc*D + c] = wo.T[kc*128 + p, c]
    woT = np.ascontiguousarray(
        wo32.T.astype(np.float16).reshape(KC, P, D).transpose(1, 0, 2)
        .reshape(P, KC * D))

    def _sw(w, sl):
        # device layout [P, D]: t[p, kc*128 + c] = w[sl][:, :].T[kc*128+p, c]
        wt = np.asarray(w, np.float32)[sl, :].T.astype(np.float16)
        return np.ascontiguousarray(
            wt.reshape(KC, P, DL).transpose(1, 0, 2).reshape(P, D))
    # bv folds into the output-projection bias: out = attn0 @ wo.T + (wo@bv+bo)
    bo_eff = np.asarray(bo, np.float32) + wo32 @ np.asarray(bv, np.float32)
    bo_t = np.ascontiguousarray(bo_eff.reshape(KC, P).T)
    maps = []
    for i in range(NCORES):
        sl = slice(i * DL, (i + 1) * DL)
        maps.append({
            "xT": xT,
            "wqT": _sw(wq, sl),
            "wkT": _sw(wk, sl),
            "wvT": _sw(wv, sl),
            "woT": woT,
            "bqk": np.ascontiguousarray(np.stack(
                [np.asarray(bq, np.float32)[sl],
                 np.asarray(bk, np.float32)[sl]], axis=1)),
            "bo_t": bo_t,
        })
    return maps


def kernel(x, wq, bq, wk, bk, wv, bv, wo, bo, **_):
    nc = _get_program()
    res = run_bass_kernel_spmd(nc, _in_maps(x, wq, bq, wk, bk, wv, bv, wo, bo),
                               list(range(NCORES)))
    # core j holds output columns [b*2048 + sh*1024 + j*128, +128) of out.T
    # at local columns b*256 + sh*128
    outT = np.empty((D, R), np.float32)
    for j in range(NCORES):
        o = res.results[j]["out"]
        for b in range(B):
            for sh in range(2):
                outT[:, b * S + sh * SH + j * CW:
                     b * S + sh * SH + (j + 1) * CW] = \
                    o[:, b * 256 + sh * CW:b * 256 + (sh + 1) * CW]
    return np.ascontiguousarray(outT.T).reshape(B, S, D)


# revision 39
# speedup vs baseline: 1.1323x; 1.0027x over previous
"""Multi-head attention (B=2, S=2048, D=1024, H=16) on 8 trn2 NeuronCores.

Tensor-parallel over heads (2 heads per core, column-sliced wq/wk/wv) for the
QKV projections and attention; a per-(batch, s-half) AllToAll redistributes
the attention output so each core computes the output projection for its own
interleaved 128-col chunks of the flattened (B*S) sequence.

Schedule (single PE stream, ACT does exp only, DVE does all copies/normalize):
  - x streams in nb-major 1MB chunks so the first matmul starts ~4us in;
    K projects first so logits/exp can start early
  - V is computed directly in natural [t, dl] layout (stationary x-chunks,
    moving wv) so no PE transposes are needed; bv folds into bo on the host
  - logits are computed transposed [t, s]; exp (ACT) feeds the P@V matmul;
    ones-columns interleaved with V produce softmax denominators in the same
    PV matmuls; normalize reads PV psum directly (reciprocal+mul on DVE)
  - batch-1 QKV interleaves into batch-0's ACT-paced attention window as PE
    filler; batch-0's output projection fills batch-1's second-half window
  - AllToAlls fire per (batch, s-half) as soon as both heads normalize; only
    the last collective plus one 128-col projection chunk remain in the tail
"""

import sys

sys.path.insert(0, "/opt/trn_rl_repo")

import numpy as np

import concourse.mybir as mybir
import concourse.tile as tile
from concourse import bacc
from concourse.bass_utils import run_bass_kernel_spmd

B, S, D = 2, 2048, 1024
H, HD = 16, 64
NCORES = 8
DL = D // NCORES          # 128 local attn dims (2 heads) per core
R = B * S                 # 4096 flattened rows
RSL = R // NCORES         # 512 output rows per core
P = 128
KC = D // P               # 8 contraction chunks of 128
TC = S // P               # 16 key/t chunks per batch
SB = 512                  # moving-operand (N) tile for QKV / logits
NB = S // SB              # 4 nb chunks per batch half of x
SH = S // 2               # 1024-wide s half
CW = 128                  # per-core output column chunk (per batch, sh)
F32 = mybir.dt.float32
F32R = mybir.dt.float32r
F16 = mybir.dt.float16

_CACHE = {}


def _build(n_iters=1, phases=3, bench=False):
    nc = bacc.Bacc("TRN2", target_bir_lowering=False, debug=False,
                   num_devices=NCORES)
    Exp = mybir.ActivationFunctionType.Exp

    kind = "Internal" if bench else "ExternalInput"
    xT = nc.dram_tensor("xT", [D, R], F16, kind=kind)
    wqT = nc.dram_tensor("wqT", [P, D], F16, kind=kind)
    wkT = nc.dram_tensor("wkT", [P, D], F16, kind=kind)
    wvT = nc.dram_tensor("wvT", [P, D], F16, kind=kind)
    woT = nc.dram_tensor("woT", [P, KC * D], F16, kind=kind)
    bqk = nc.dram_tensor("bqk", [DL, 2], F32, kind=kind)
    bo_t = nc.dram_tensor("bo_t", [P, KC], F32, kind=kind)
    out = nc.dram_tensor("out", [D, RSL], F32, kind="ExternalOutput")

    with tile.TileContext(nc) as tc:
        with (
            tc.tile_pool(name="const", bufs=1) as const,
            tc.tile_pool(name="persist", bufs=1) as persist,
            tc.tile_pool(name="dram", bufs=1, space="DRAM") as dram,
        ):
            bias2 = const.tile([DL, 2], F32, tag="bias2")
            bo_s = const.tile([P, KC], F32, tag="bo_s")
            w_s = {}
            for name in ("wq", "wk", "wv"):
                w_s[name] = const.tile([P, D], F16, tag=f"w_{name}",
                                       name=f"w_{name}")
            wo_all = const.tile([P, KC * D], F16, tag="wo_all")
            wo_s = [wo_all[:, kc * D:(kc + 1) * D] for kc in range(KC)]

            def load_w(t, wt):
                if bench:
                    nc.vector.memset(t[:], 0.0)
                else:
                    nc.sync.dma_start(t[:], wt[:, :])

            # wk first: the K projection runs first so exp starts early
            load_w(w_s["wk"], wkT)

            # persistent activations
            QT = persist.tile([P, R], F16, tag="QT")   # [2 heads*64, B*S]
            KT = persist.tile([P, R], F16, tag="KT")
            # V natural per 128-row t-chunk: [v_h0 |ones| v_h1 |ones]
            vn = persist.tile([P, (R // P) * 256], F16, tag="vn")
            vn3 = vn[:].rearrange("p (g two c) -> p g two c", two=2, c=128)
            nc.vector.memset(vn3[:, :, :, 64:128], 1.0)
            attnTs = [persist.tile([P, R], F16, tag=f"attnT{h}",
                                   name=f"attnT{h}") for h in range(2)]

            for it in range(n_iters):
                a2a_in = [[dram.tile([NCORES, P, CW], F16,
                                     tag=f"a2a_in{it}_{b}_{sh}",
                                     name=f"a2a_in{it}_{b}_{sh}")
                           for sh in range(2)] for b in range(B)]
                a2a_out = [[dram.tile([NCORES, P, CW], F16,
                                      tag=f"a2a_out{it}_{b}_{sh}",
                                      name=f"a2a_out{it}_{b}_{sh}")
                            for sh in range(2)] for b in range(B)]

                with tc.tile_pool(name=f"xt{it}", bufs=2) as xt_pool:
                    xth = [xt_pool.tile([P, KC * S], F16, tag="xt",
                                        name=f"xt_{it}_{half}")
                           for half in range(2)]

                    def xs(half, kc):
                        return xth[half][:, kc * S:(kc + 1) * S]

                    def load_x(half):
                        # nb-major so the first 1MB arrives fast; one
                        # strided DMA per nb covers all kc chunks
                        hof = half * S
                        xv = xth[half][:].rearrange("p (kc s) -> p kc s", s=S)
                        srcv = xT.rearrange("(kc p) r -> p kc r", p=P)
                        for nb in range(NB):
                            nsplit = 4 if (half == 0 and nb == 0) else 2
                            for kh in range(nsplit):
                                kq = KC // nsplit
                                ks = slice(kh * kq, (kh + 1) * kq)
                                nc.sync.dma_start(
                                    xv[:, ks, nb * SB:(nb + 1) * SB],
                                    srcv[:, ks,
                                         hof + nb * SB:hof + (nb + 1) * SB])
                            if half == 0 and nb == 0:
                                load_w(w_s["wq"], wqT)
                                load_w(w_s["wv"], wvT)
                                if bench:
                                    nc.vector.memset(bias2[:], 0.0)
                                    nc.vector.memset(bo_s[:], 0.0)
                                else:
                                    nc.sync.dma_start(bias2[:], bqk[:])
                                    nc.sync.dma_start(bo_s[:], bo_t[:])

                    def qk_group(pj, half, nb, pool, tag):
                        # one [128dl, 512rows] psum group + DVE copy w/ bias
                        name, dst = (("wk", KT), ("wq", QT))[pj]
                        ps = pool.tile([P, SB], F32, tag=tag,
                                       name=f"qk_{it}_{half}_{pj}_{nb}")
                        w = w_s[name]
                        for kc in range(KC):
                            nc.tensor.matmul(
                                ps[:], w[:, kc * P:(kc + 1) * P],
                                xs(half, kc)[:, nb * SB:(nb + 1) * SB],
                                start=(kc == 0), stop=(kc == KC - 1))
                        o = half * S + nb * SB
                        nc.vector.tensor_scalar_add(
                            dst[:, o:o + SB], ps[:],
                            bias2[:, 1 - pj:2 - pj])

                    def v_group(half, q, pool, tag):
                        # natural-V for 4 t-chunks: psum [128t, 4*128dl]
                        ps = pool.tile([P, SB], F32, tag=tag,
                                       name=f"vq_{it}_{half}_{q}")
                        wv = w_s["wv"]
                        for i in range(4):
                            tcn = q * 4 + i
                            for kc in range(KC):
                                nc.tensor.matmul(
                                    ps[:, i * P:(i + 1) * P],
                                    xs(half, kc)[:, tcn * P:(tcn + 1) * P],
                                    wv[:, kc * P:(kc + 1) * P],
                                    start=(kc == 0), stop=(kc == KC - 1))
                        for i in range(4):
                            g = half * TC + q * 4 + i
                            o = g * 256
                            nc.vector.tensor_copy(
                                vn[:, o:o + 64], ps[:, i * P:i * P + 64])
                            nc.vector.tensor_copy(
                                vn[:, o + 128:o + 192],
                                ps[:, i * P + 64:(i + 1) * P])

                    norm_muls = {}

                    def attention_batch(b, ps3, exps, norm):
                        base = b * S
                        for sh in range(2):
                            for h in range(2):
                                hr = slice(h * HD, (h + 1) * HD)
                                sof = base + sh * SH
                                pv = ps3.tile([P, SH], F32, tag="pv", bufs=1,
                                              name=f"pv_{it}_{b}_{h}_{sh}")
                                for tcn in range(TC):
                                    ex = exps.tile(
                                        [P, SH], F16, tag="ex",
                                        name=f"ex_{it}_{b}_{h}_{sh}_{tcn}")
                                    lg = ps3.tile(
                                        [P, SH], F32, tag="lg", bufs=2,
                                        name=f"lg_{it}_{b}_{h}_{sh}_{tcn}")
                                    with tc.high_priority(offset=400):
                                        for sb in range(2):
                                            nc.tensor.matmul(
                                                lg[:, sb * SB:(sb + 1) * SB],
                                                KT[hr, base + tcn * P:
                                                   base + (tcn + 1) * P],
                                                QT[hr, sof + sb * SB:
                                                   sof + (sb + 1) * SB],
                                                start=True, stop=True)
                                        nc.scalar.activation(
                                            ex[:], lg[:], Exp,
                                            scale=1.0 / 8.0)
                                        o = (b * TC + tcn) * 256 + h * 128
                                        for sb in range(2):
                                            nc.tensor.matmul(
                                                pv[:, sb * SB:(sb + 1) * SB],
                                                vn[:, o:o + 128],
                                                ex[:, sb * SB:
                                                   (sb + 1) * SB],
                                                start=(tcn == 0),
                                                stop=(tcn == TC - 1))
                                # normalize straight out of PV psum,
                                # per sb half so it pipelines with pv mms
                                rc = norm.tile([HD, SH], F32, tag="rc")
                                nq, qw = 2, SB
                                with tc.high_priority(offset=400):
                                    for sb in range(nq):
                                        sl = slice(sb * qw, (sb + 1) * qw)
                                        nc.vector.reciprocal(
                                            rc[:, sl], pv[64:128, sl])
                                        norm_muls[(b, sh, h, sb)] = \
                                            nc.vector.tensor_mul(
                                                attnTs[h][0:HD,
                                                          sof + sb * qw:
                                                          sof + (sb + 1) * qw],
                                                pv[0:64, sl], rc[:, sl])
                            # ship this (batch, s-half); overlaps compute
                            if phases >= 3:
                                stk = tc.high_priority(offset=400)
                                stk.__enter__()
                                for h2 in (1, 0):
                                    hs = slice(h2 * HD, (h2 + 1) * HD)
                                    nc.sync.dma_start(
                                        a2a_in[b][sh].rearrange(
                                            "j p c -> p j c")[hs],
                                        attnTs[h2][0:HD,
                                                   base + sh * SH:
                                                   base + (sh + 1) * SH]
                                        .rearrange("p (j c) -> p j c", c=CW))
                                nc.gpsimd.collective_compute(
                                    "AllToAll", mybir.AluOpType.bypass,
                                    replica_groups=[list(range(NCORES))],
                                    ins=[a2a_in[b][sh].opt()],
                                    outs=[a2a_out[b][sh].opt()])
                                stk.__exit__(None, None, None)

                    outv = out.rearrange("(mc p) c -> p mc c", p=P)

                    def proj_pass(b, sh, rh, ncols, coff, ps4, outs):
                        # 8 mc-chunks + one batched out DMA
                        ot = outs.tile([P, KC * ncols], F32, tag="ot",
                                       name=f"ot_{it}_{b}_{sh}")
                        for mc in range(KC):
                            ps = ps4.tile([P, SB], F32, tag="aux",
                                          name=f"ps4_{it}_{b}_{sh}_{mc}")
                            for kc in range(KC):
                                nc.tensor.matmul(
                                    ps[:, 0:ncols],
                                    wo_s[kc][:, mc * P:(mc + 1) * P],
                                    rh[:, kc * ncols:(kc + 1) * ncols],
                                    start=(kc == 0), stop=(kc == KC - 1))
                            nc.vector.tensor_scalar_add(
                                ot[:, mc * ncols:(mc + 1) * ncols],
                                ps[:, 0:ncols], bo_s[:, mc:mc + 1])
                        for mh in range(2):
                            ms = slice(mh * (KC // 2), (mh + 1) * (KC // 2))
                            nc.sync.dma_start(
                                outv[:, ms, coff:coff + ncols],
                                ot[:].rearrange("p (mc c) -> p mc c",
                                                c=ncols)[:, ms])

                    # ---- head: QKV-b0, K first, V natural; the attn
                    # psum pools coexist so the first logits/exp can be
                    # hoisted under the head by the scheduler ----
                    with (
                        tc.tile_pool(name=f"ps3{it}", bufs=1,
                                     space="PSUM") as ps3,
                        tc.tile_pool(name=f"exps{it}", bufs=16) as exps,
                        tc.tile_pool(name=f"norm{it}", bufs=2) as norm,
                        tc.tile_pool(name=f"aux{it}", bufs=2,
                                     space="PSUM") as aux,
                    ):
                        load_x(0)
                        qk_group(0, 0, 0, aux, "aux")  # K nb0
                        qk_group(1, 0, 0, aux, "aux")  # Q nb0
                        qk_group(1, 0, 1, aux, "aux")  # Q nb1
                        qk_group(0, 0, 1, aux, "aux")  # K nb1
                        qk_group(0, 0, 2, aux, "aux")  # K nb2
                        qk_group(0, 0, 3, aux, "aux")  # K nb3
                        for q in range(4):
                            v_group(0, q, aux, "aux")
                        qk_group(1, 0, 2, aux, "aux")  # Q nb2
                        qk_group(1, 0, 3, aux, "aux")  # Q nb3
                        load_x(1)

                        if bench:
                            nc.vector.memset(wo_all[:], 0.0)
                        else:
                            nc.sync.dma_start(wo_all[:], woT[:, :])
                        if phases < 2:
                            continue

                        attention_batch(0, ps3, exps, norm)

                        # fillers drain into attention's PE slack:
                        # K/Q-s0 of b1 gate attn-b1's logits; V-b1 and
                        # Q-s1-b1 defer into the b1 window
                        for pj, nb in ((0, 0), (1, 0), (1, 1), (0, 1),
                                       (0, 2), (0, 3)):
                            qk_group(pj, 1, nb, aux, "aux")
                        for q in range(4):
                            v_group(1, q, aux, "aux")
                        qk_group(1, 1, 2, aux, "aux")
                        qk_group(1, 1, 3, aux, "aux")

                        if phases < 3:
                            continue

                        attention_batch(1, ps3, exps, norm)

                        # output projection, one pass per (batch, s-half).
                        # rh DMAs gated (dep=) so the scheduler cannot hoist
                        # dependent matmuls before the collective really
                        # lands; ungated passes hoist into attn-b1 PE gaps.
                        with (
                            tc.tile_pool(name=f"proj{it}", bufs=1) as proj,
                            tc.tile_pool(name=f"outs{it}", bufs=2) as outs,
                        ):
                            # all but the last pass run under the final
                            # collective: gate their rh loads on the last
                            # norm so the scheduler cannot pull the matmuls
                            # into the (already saturated) attention windows
                            import bass_rust as _br
                            last_norm = norm_muls[(1, 1, 1, 1)]
                            for b in range(B):
                                for sh in range(2):
                                    rh = proj.tile([P, KC * CW], F16,
                                                   tag=f"rh{it}_{b}_{sh}",
                                                   name=f"rh{it}_{b}_{sh}")
                                    for kh in range(2):
                                        ks = slice(kh * (KC // 2),
                                                   (kh + 1) * (KC // 2))
                                        dma = nc.sync.dma_start(
                                            rh[:].rearrange(
                                                "p (kc c) -> p kc c",
                                                c=CW)[:, ks],
                                            a2a_out[b][sh].rearrange(
                                                "kc p c -> p kc c")[:, ks])
                                        if (b, sh) != (1, 1):
                                            dma.ins.add_dependency(
                                                last_norm.ins.name,
                                                _br.DependencyInfo(
                                                    sync=True,
                                                    no_sync=False))
                                    proj_pass(b, sh, rh, CW,
                                              b * 2 * CW + sh * CW,
                                              aux, outs)
    nc.compile()
    return nc


def _get_program(n_iters=1, phases=3, bench=False):
    key = (n_iters, phases, bench)
    if key not in _CACHE:
        _CACHE[key] = _build(n_iters, phases, bench)
    return _CACHE[key]


def _in_maps(x, wq, bq, wk, bk, wv, bv, wo, bo):
    x = np.asarray(x, np.float32)
    xT = np.ascontiguousarray(x.reshape(R, D).T.astype(np.float16))
    wo32 = np.asarray(wo, np.float32)
    # device layout [P, KC*D]: woT_sw[p, kc*D + c] = wo.T[kc*128 + p, c]
    woT = np.ascontiguousarray(
        wo32.T.astype(np.float16).reshape(KC, P, D).transpose(1, 0, 2)
        .reshape(P, KC * D))

    def _sw(w, sl):
        # device layout [P, D]: t[p, kc*128 + c] = w[sl][:, :].T[kc*128+p, c]
        wt = np.asarray(w, np.float32)[sl, :].T.astype(np.float16)
        return np.ascontiguousarray(
            wt.reshape(KC, P, DL).transpose(1, 0, 2).reshape(P, D))
    # bv folds into the output-projection bias: out = attn0 @ wo.T + (wo@bv+bo)
    bo_eff = np.asarray(bo, np.float32) + wo32 @ np.asarray(bv, np.float32)
    bo_t = np.ascontiguousarray(bo_eff.reshape(KC, P).T)
    maps = []
    for i in range(NCORES):
        sl = slice(i * DL, (i + 1) * DL)
        maps.append({
            "xT": xT,
            "wqT": _sw(wq, sl),
            "wkT": _sw(wk, sl),
            "wvT": _sw(wv, sl),
            "woT": woT,
            "bqk": np.ascontiguousarray(np.stack(
                [np.asarray(bq, np.float32)[sl],
                 np.asarray(bk, np.float32)[sl]], axis=1)),
            "bo_t": bo_t,
        })
    return maps


def kernel(x, wq, bq, wk, bk, wv, bv, wo, bo, **_):
    nc = _get_program()
    res = run_bass_kernel_spmd(nc, _in_maps(x, wq, bq, wk, bk, wv, bv, wo, bo),
                               list(range(NCORES)))
    # core j holds output columns [b*2048 + sh*1024 + j*128, +128) of out.T
    # at local columns b*256 + sh*128
    outT = np.empty((D, R), np.float32)
    for j in range(NCORES):
        o = res.results[j]["out"]
        for b in range(B):
            for sh in range(2):
                outT[:, b * S + sh * SH + j * CW:
                     b * S + sh * SH + (j + 1) * CW] = \
                    o[:, b * 256 + sh * CW:b * 256 + (sh + 1) * CW]
    return np.ascontiguousarray(outT.T).reshape(B, S, D)
